# revision 25
# baseline (speedup 1.0000x reference)
"""Trainium2 Bass kernel for nn_LIMADNN2_42013370090068 (dense_mlp).

Reference semantics: out depends only on x[:, 0, :] — the `state.add(...)`
neighbor loop in the torch module is not in-place, so the 65-neighbor
dimension is dead. force_prev = x[:, 0, 6:9] is a pure slice.

  q   = x[:, 0, :]                 # [B, 12]
  h   = relu(q @ W1 + b1)          # [B, 16]
  blk = relu(h @ W2 + b2)          # [B, 8]
  out = (blk @ Ws + bs) @ Wo + bo  # [B, 3]   (no relu between -> folded)

Device strategy (pure data parallel, 8 cores, batch-sharded, fp16):
  * Host slices q (12.6 MB of the 818 MB input), casts fp16; all matmul
    operands fp16 (1 cyc/col on PE vs 4 for fp32).
  * Full-width block-diagonal matmuls stream atoms on the free dim at
    the per-layer packing limit: L1 [96x128] = 8 chunks/stream,
    L2 [128x64] col-paired = 8/stream, L3 [128x48] col-paired =
    16/stream -> 20 matmuls/core, 10240 streamed columns total (the PE
    floor for this shape). Back-to-back emission keeps the PE busy so
    the HAM clock-gate ramps to 2.4 GHz mid-kernel.
  * One 8-bank PSUM tile, manually sliced: L1 -> banks 0-7, L2 reuses
    0-3, L3 reuses 4-5 (range-tracked WAR via drains).
  * PSUM drains (only ScalarE+VectorE reach PSUM, 1 col/cyc fp32) are
    split across both engines, bias+relu fused.
  * A dummy relu at t=0 prefetches the ACT spline table (~1.3 us) under
    the fixed preamble.
"""

import numpy as np

B = 262144
F = 12
N_CORES = 8
BPC = B // N_CORES          # 32768 atoms per core
TN = 512                    # atoms per chunk == matmul free dim == psum bank
NP = 8                      # L1 passes (8 chunks of 8 atoms-chunks each)

W1OFF = 0                   # wpack column offsets
W2OFF = 128
W3OFF = 192                 # block a at 192, block b at 256


def _build_nc():
    import concourse.tile as tile
    from concourse import bacc, mybir

    f16 = mybir.dt.float16
    f32 = mybir.dt.float32

    nc = bacc.Bacc("TRN2", target_bir_lowering=False, debug=False,
                   num_devices=N_CORES)

    xin = nc.dram_tensor("xin", [96, 4096], f16, kind="ExternalInput")
    wpack = nc.dram_tensor("wpack", [128, 312], f16, kind="ExternalInput")
    out = nc.dram_tensor("out", [112, 1024], f16, kind="ExternalOutput")

    Relu = mybir.ActivationFunctionType.Relu
    Ident = mybir.ActivationFunctionType.Identity
    add, vmax = mybir.AluOpType.add, mybir.AluOpType.max

    with tile.TileContext(nc) as tc:
        with (
            tc.tile_pool(name="const", bufs=1) as cpool,
            tc.tile_pool(name="x", bufs=1) as xpool,
            tc.tile_pool(name="h", bufs=1) as hpool,
            tc.tile_pool(name="blk", bufs=1) as bpool,
            tc.tile_pool(name="osb", bufs=1) as opool,
            tc.tile_pool(name="ps", bufs=1, space="PSUM") as pspool,
        ):
            wsb = cpool.tile([128, 312], f16)
            scrm = cpool.tile([96, 512], f16)       # PE warm-up fodder
            bview = wsb[0:128, 304:312].bitcast(f32)  # fp32 biases, packed
            b1a = bview[0:128, 0:1]
            b2a = bview[0:128, 1:2]
            boa = bview[0:112, 2:3]

            xsb = xpool.tile([96, 4096], f16)
            hsb = hpool.tile([128, 4096], f16)
            blksb = bpool.tile([128, 2048], f16)
            osb = opool.tile([112, 1024], f16)
            ps = pspool.tile([128, 4096], f32)      # all 8 banks

            # Warm the PE HAM clock-gate (~3.4us of sustained matmuls)
            # on zero scratch while the input DMAs are still in flight.
            # (The ACT table load is auto-hoisted to the Scalar stream
            # start, so no dummy activation is needed.)
            nc.vector.memset(scrm[:].bitcast(mybir.dt.uint32), 0)
            for _ in range(13):
                nc.tensor.matmul(ps[0:128, 3584:4096],
                                 scrm[0:96, 0:128],
                                 scrm[0:96, 0:512],
                                 start=True, stop=True)

            # Input DMAs: x on Sync + GpSimd queues, weights alone on
            # Scalar (its HWDGE ring also carries the ACT table load).
            nc.scalar.dma_start(wsb[:], wpack[:])
            nc.sync.dma_start(xsb[:, 0:1024], xin[:, 0:1024])
            nc.sync.dma_start(xsb[:, 1024:2048], xin[:, 1024:2048])
            nc.sync.dma_start(xsb[:, 2048:3072], xin[:, 2048:3072])
            nc.gpsimd.dma_start(xsb[:, 3072:4096], xin[:, 3072:4096])

            # L1: 8 full-width matmuls, 8 chunks each -> banks 0-7.
            for p in range(NP):
                nc.tensor.matmul(ps[0:128, TN * p:TN * p + TN],
                                 wsb[0:96, W1OFF:W1OFF + 128],
                                 xsb[0:96, TN * p:TN * p + TN],
                                 start=True, stop=True)

            # relu1 drains: per-bank alternating Scalar/Vector; the last
            # bank is split across both engines (tail latency).
            for d in range(7):
                lo = TN * d
                if d % 2 == 0:
                    nc.scalar.activation(hsb[:, lo:lo + TN],
                                         ps[:, lo:lo + TN],
                                         Relu, bias=b1a)
                else:
                    nc.vector.tensor_scalar(hsb[:, lo:lo + TN],
                                            ps[:, lo:lo + TN],
                                            b1a, 0.0, add, vmax)
            nc.vector.tensor_scalar(hsb[:, 3584:3840], ps[:, 3584:3840],
                                    b1a, 0.0, add, vmax)
            nc.scalar.activation(hsb[:, 3840:4096], ps[:, 3840:4096],
                                 Relu, bias=b1a)

            # L2: 4 passes x 2 col-paired matmuls -> banks 0-3 (reused).
            for q in range(4):
                for e in range(2):
                    nc.tensor.matmul(
                        ps[64 * e:64 * e + 64, TN * q:TN * q + TN],
                        wsb[0:128, W2OFF:W2OFF + 64],
                        hsb[0:128, TN * (2 * q + e):TN * (2 * q + e) + TN],
                        start=True, stop=True)

            # relu2 drains: per-bank alternating; last bank split.
            for d in range(3):
                lo = TN * d
                if d % 2 == 0:
                    nc.scalar.activation(blksb[:, lo:lo + TN],
                                         ps[:, lo:lo + TN],
                                         Relu, bias=b2a)
                else:
                    nc.vector.tensor_scalar(blksb[:, lo:lo + TN],
                                            ps[:, lo:lo + TN],
                                            b2a, 0.0, add, vmax)
            nc.vector.tensor_scalar(blksb[:, 1536:1792], ps[:, 1536:1792],
                                    b2a, 0.0, add, vmax)
            nc.scalar.activation(blksb[:, 1792:2048], ps[:, 1792:2048],
                                 Relu, bias=b2a)

            # L3: 2 passes x 2 col-paired matmuls -> banks 4-5 (reused).
            for r in range(2):
                for m in range(2):
                    nc.tensor.matmul(
                        ps[64 * m:64 * m + 48,
                           TN * (4 + r):TN * (4 + r) + TN],
                        wsb[0:128, W3OFF + 64 * m:W3OFF + 64 * m + 48],
                        blksb[0:128, TN * (2 * r + m):TN * (2 * r + m) + TN],
                        start=True, stop=True)

            # out drains (+bso): 256-col pieces split Scalar/Vector, then
            # the store DMA per 512-col bank.
            for r in range(2):
                po = TN * (4 + r)
                oo = TN * r
                nc.scalar.activation(osb[0:112, oo:oo + 256],
                                     ps[0:112, po:po + 256],
                                     Ident, bias=boa)
                nc.vector.tensor_scalar(osb[0:112, oo + 256:oo + TN],
                                        ps[0:112, po + 256:po + TN],
                                        boa, None, add)
                nc.sync.dma_start(out[0:112, oo:oo + TN],
                                  osb[0:112, oo:oo + TN])

    nc.finalize()
    return nc


def _host_prep(x, W1, b1, W2, b2, Ws, bs, Wo, bo):
    x = np.asarray(x)
    W1 = np.asarray(W1, dtype=np.float32)
    b1 = np.asarray(b1, dtype=np.float32)
    W2 = np.asarray(W2, dtype=np.float32)
    b2 = np.asarray(b2, dtype=np.float32)
    Ws = np.asarray(Ws, dtype=np.float32)
    bs = np.asarray(bs, dtype=np.float32)
    Wo = np.asarray(Wo, dtype=np.float32)
    bo = np.asarray(bo, dtype=np.float32)

    q = np.ascontiguousarray(x[:, 0, :], dtype=np.float32)       # [B, 12]
    force_prev = np.ascontiguousarray(x[:, 0, 6:9], dtype=np.float32)

    # Fold the two linear layers that have no nonlinearity between them.
    Wso = (Ws.astype(np.float64) @ Wo.astype(np.float64)).astype(np.float32)
    bso = (bs.astype(np.float64) @ Wo.astype(np.float64)
           + bo.astype(np.float64)).astype(np.float32)

    W1h = W1.astype(np.float16)
    W2h = W2.astype(np.float16)
    W3h = Wso.astype(np.float16)

    wpack = np.zeros((128, 312), np.float16)
    for c8 in range(8):                             # L1: 8-chunk block-diag
        wpack[12 * c8:12 * c8 + 12,
              W1OFF + 16 * c8:W1OFF + 16 * c8 + 16] = W1h
    for c8 in range(8):                             # L2: 8-chunk block-diag
        wpack[16 * c8:16 * c8 + 16,
              W2OFF + 8 * c8:W2OFF + 8 * c8 + 8] = W2h
    w3b = np.zeros((128, 48), np.float16)           # L3: 16-chunk block-diag
    for e in range(2):
        for c8 in range(8):
            t = 8 * e + c8
            w3b[64 * e + 8 * c8:64 * e + 8 * c8 + 8,
                3 * t:3 * t + 3] = W3h
    wpack[:, W3OFF:W3OFF + 48] = w3b
    wpack[:, W3OFF + 64:W3OFF + 112] = w3b

    bias32 = np.zeros((128, 4), np.float32)
    bias32[:, 0] = np.tile(b1, 8)
    bias32[:, 1] = np.tile(b2, 16)
    bias32[0:48, 2] = np.tile(bso, 16)
    bias32[64:112, 2] = np.tile(bso, 16)
    wpack[:, 304:312] = bias32.view(np.float16)

    in_maps = []
    for c in range(N_CORES):
        qc = q[c * BPC:(c + 1) * BPC].astype(np.float16)
        # chunk c = 8p + c8 (512 atoms); [row = 12*c8 + f, col = 512p + a]
        t = qc.reshape(NP, 8, TN, F)                # p c8 a f
        xc = np.ascontiguousarray(
            t.transpose(1, 3, 0, 2).reshape(96, 4096))
        in_maps.append({"xin": xc, "wpack": wpack})
    return in_maps, force_prev


def _host_gather(results):
    out = np.empty((B, 3), np.float32)
    for c in range(N_CORES):
        Oc = results[c]["out"]                      # [112, 1024] fp16
        oc = np.empty((64, TN, 3), np.float32)
        for r in range(2):
            for m in range(2):
                blkO = Oc[64 * m:64 * m + 48,
                          TN * r:TN * r + TN].astype(np.float32)
                # row = 3t + rr, t = 8e + c8; chunk = 32r + 16m + t
                oc[32 * r + 16 * m:32 * r + 16 * m + 16] = (
                    blkO.reshape(16, 3, TN).transpose(0, 2, 1))
        out[c * BPC:(c + 1) * BPC] = oc.reshape(BPC, 3)
    return out


LAST_RESULT = None


def kernel(x, W1, b1, W2, b2, Ws, bs, Wo, bo):
    from concourse.bass_utils import run_bass_kernel_spmd

    in_maps, force_prev = _host_prep(x, W1, b1, W2, b2, Ws, bs, Wo, bo)
    nc = _build_nc()
    res = run_bass_kernel_spmd(nc, in_maps, core_ids=list(range(N_CORES)))
    globals()["LAST_RESULT"] = res
    out = _host_gather(res.results)
    return (out, force_prev)


# revision 27
# speedup vs baseline: 1.0259x; 1.0259x over previous
"""Trainium2 Bass kernel for nn_LIMADNN2_42013370090068 (dense_mlp).

Reference semantics: out depends only on x[:, 0, :] — the `state.add(...)`
neighbor loop in the torch module is not in-place, so the 65-neighbor
dimension is dead. force_prev = x[:, 0, 6:9] is a pure slice.

  q   = x[:, 0, :]                 # [B, 12]
  h   = relu(q @ W1 + b1)          # [B, 16]
  blk = relu(h @ W2 + b2)          # [B, 8]
  out = (blk @ Ws + bs) @ Wo + bo  # [B, 3]   (no relu between -> folded)

Device strategy (pure data parallel, 8 cores, batch-sharded, fp16):
  * Host slices q (12.6 MB of the 818 MB input), casts fp16; all matmul
    operands fp16 (1 cyc/col on PE vs 4 for fp32).
  * Full-width block-diagonal matmuls stream atoms on the free dim at
    the per-layer packing limit: L1 [96x128] = 8 chunks/stream,
    L2 [128x64] col-paired = 8/stream, L3 [128x48] col-paired =
    16/stream -> 20 matmuls/core, 10240 streamed columns total (the PE
    floor for this shape). Back-to-back emission keeps the PE busy so
    the HAM clock-gate ramps to 2.4 GHz mid-kernel.
  * One 8-bank PSUM tile, manually sliced: L1 -> banks 0-7, L2 reuses
    0-3, L3 reuses 4-5 (range-tracked WAR via drains).
  * PSUM drains (only ScalarE+VectorE reach PSUM, 1 col/cyc fp32) are
    split across both engines, bias+relu fused.
  * A dummy relu at t=0 prefetches the ACT spline table (~1.3 us) under
    the fixed preamble.
"""

import numpy as np

B = 262144
F = 12
N_CORES = 8
BPC = B // N_CORES          # 32768 atoms per core
TN = 512                    # atoms per chunk == matmul free dim == psum bank
NP = 8                      # L1 passes (8 chunks of 8 atoms-chunks each)

W1OFF = 0                   # wpack column offsets
W2OFF = 128
W3OFF = 192                 # block a at 192, block b at 256


def _build_nc():
    import concourse.tile as tile
    from concourse import bacc, mybir

    f16 = mybir.dt.float16
    f32 = mybir.dt.float32

    nc = bacc.Bacc("TRN2", target_bir_lowering=False, debug=False,
                   num_devices=N_CORES)

    xin = nc.dram_tensor("xin", [96, 4096], f16, kind="ExternalInput")
    wpack = nc.dram_tensor("wpack", [128, 312], f16, kind="ExternalInput")
    out = nc.dram_tensor("out", [112, 1024], f16, kind="ExternalOutput")

    Relu = mybir.ActivationFunctionType.Relu
    Ident = mybir.ActivationFunctionType.Identity
    add, vmax = mybir.AluOpType.add, mybir.AluOpType.max

    with tile.TileContext(nc) as tc:
        with (
            tc.tile_pool(name="const", bufs=1) as cpool,
            tc.tile_pool(name="x", bufs=1) as xpool,
            tc.tile_pool(name="h", bufs=1) as hpool,
            tc.tile_pool(name="blk", bufs=1) as bpool,
            tc.tile_pool(name="osb", bufs=1) as opool,
            tc.tile_pool(name="ps", bufs=1, space="PSUM") as pspool,
        ):
            wsb = cpool.tile([128, 312], f16)
            scrm = cpool.tile([96, 512], f16)       # PE warm-up fodder
            bview = wsb[0:128, 304:312].bitcast(f32)  # fp32 biases, packed
            b1a = bview[0:128, 0:1]
            b2a = bview[0:128, 1:2]
            boa = bview[0:112, 2:3]

            xsb = xpool.tile([96, 4096], f16)
            hsb = hpool.tile([128, 4096], f16)
            blksb = bpool.tile([128, 2048], f16)
            osb = opool.tile([112, 1024], f16)
            ps = pspool.tile([128, 4096], f32)      # all 8 banks

            # Warm the PE HAM clock-gate (~3.4us of sustained matmuls)
            # on zero scratch while the input DMAs are still in flight.
            # (The ACT table load is auto-hoisted to the Scalar stream
            # start, so no dummy activation is needed.)
            nc.vector.memset(scrm[:].bitcast(mybir.dt.uint32), 0)

            def dummy_mm(cols):
                nc.tensor.matmul(ps[0:128, 3584:3584 + cols],
                                 scrm[0:96, 0:128],
                                 scrm[0:96, 0:cols],
                                 start=True, stop=True)

            for _ in range(7):
                dummy_mm(512)

            # Input DMAs: x on Sync + GpSimd queues, weights alone on
            # Scalar (its HWDGE ring also carries the ACT table load).
            nc.scalar.dma_start(wsb[:], wpack[:])
            nc.sync.dma_start(xsb[:, 0:1024], xin[:, 0:1024])
            nc.sync.dma_start(xsb[:, 1024:2048], xin[:, 1024:2048])
            nc.sync.dma_start(xsb[:, 2048:3072], xin[:, 2048:3072])
            nc.gpsimd.dma_start(xsb[:, 3072:4096], xin[:, 3072:4096])

            # L1: 8 full-width matmuls, 8 chunks each -> banks 0-7.
            # Small filler dummies bridge the x-block DMA arrival gaps so
            # the PE stream stays gapless (HAM stays on track to warm).
            fills = {2: 3, 4: 2, 6: 1}
            for p in range(NP):
                for _ in range(fills.get(p, 0)):
                    dummy_mm(128)
                nc.tensor.matmul(ps[0:128, TN * p:TN * p + TN],
                                 wsb[0:96, W1OFF:W1OFF + 128],
                                 xsb[0:96, TN * p:TN * p + TN],
                                 start=True, stop=True)

            # relu1 drains: per-bank alternating Scalar/Vector; the last
            # bank is split across both engines (tail latency).
            for d in range(7):
                lo = TN * d
                if d % 2 == 0:
                    nc.scalar.activation(hsb[:, lo:lo + TN],
                                         ps[:, lo:lo + TN],
                                         Relu, bias=b1a)
                else:
                    nc.vector.tensor_scalar(hsb[:, lo:lo + TN],
                                            ps[:, lo:lo + TN],
                                            b1a, 0.0, add, vmax)
            nc.vector.tensor_scalar(hsb[:, 3584:3840], ps[:, 3584:3840],
                                    b1a, 0.0, add, vmax)
            nc.scalar.activation(hsb[:, 3840:4096], ps[:, 3840:4096],
                                 Relu, bias=b1a)

            # L2: 4 passes x 2 col-paired matmuls -> banks 0-3 (reused).
            for q in range(4):
                for e in range(2):
                    nc.tensor.matmul(
                        ps[64 * e:64 * e + 64, TN * q:TN * q + TN],
                        wsb[0:128, W2OFF:W2OFF + 64],
                        hsb[0:128, TN * (2 * q + e):TN * (2 * q + e) + TN],
                        start=True, stop=True)

            # relu2 drains: per-bank alternating; last bank split.
            for d in range(3):
                lo = TN * d
                if d % 2 == 0:
                    nc.scalar.activation(blksb[:, lo:lo + TN],
                                         ps[:, lo:lo + TN],
                                         Relu, bias=b2a)
                else:
                    nc.vector.tensor_scalar(blksb[:, lo:lo + TN],
                                            ps[:, lo:lo + TN],
                                            b2a, 0.0, add, vmax)
            nc.vector.tensor_scalar(blksb[:, 1536:1792], ps[:, 1536:1792],
                                    b2a, 0.0, add, vmax)
            nc.scalar.activation(blksb[:, 1792:2048], ps[:, 1792:2048],
                                 Relu, bias=b2a)

            # L3: 2 passes x 2 col-paired matmuls -> banks 4-5 (reused).
            for r in range(2):
                for m in range(2):
                    nc.tensor.matmul(
                        ps[64 * m:64 * m + 48,
                           TN * (4 + r):TN * (4 + r) + TN],
                        wsb[0:128, W3OFF + 64 * m:W3OFF + 64 * m + 48],
                        blksb[0:128, TN * (2 * r + m):TN * (2 * r + m) + TN],
                        start=True, stop=True)

            # out drains (+bso): 256-col pieces split Scalar/Vector, then
            # the store DMA per 512-col bank.
            for r in range(2):
                po = TN * (4 + r)
                oo = TN * r
                nc.scalar.activation(osb[0:112, oo:oo + 256],
                                     ps[0:112, po:po + 256],
                                     Ident, bias=boa)
                nc.vector.tensor_scalar(osb[0:112, oo + 256:oo + TN],
                                        ps[0:112, po + 256:po + TN],
                                        boa, None, add)
                nc.sync.dma_start(out[0:112, oo:oo + TN],
                                  osb[0:112, oo:oo + TN])

    nc.finalize()
    return nc


def _host_prep(x, W1, b1, W2, b2, Ws, bs, Wo, bo):
    x = np.asarray(x)
    W1 = np.asarray(W1, dtype=np.float32)
    b1 = np.asarray(b1, dtype=np.float32)
    W2 = np.asarray(W2, dtype=np.float32)
    b2 = np.asarray(b2, dtype=np.float32)
    Ws = np.asarray(Ws, dtype=np.float32)
    bs = np.asarray(bs, dtype=np.float32)
    Wo = np.asarray(Wo, dtype=np.float32)
    bo = np.asarray(bo, dtype=np.float32)

    q = np.ascontiguousarray(x[:, 0, :], dtype=np.float32)       # [B, 12]
    force_prev = np.ascontiguousarray(x[:, 0, 6:9], dtype=np.float32)

    # Fold the two linear layers that have no nonlinearity between them.
    Wso = (Ws.astype(np.float64) @ Wo.astype(np.float64)).astype(np.float32)
    bso = (bs.astype(np.float64) @ Wo.astype(np.float64)
           + bo.astype(np.float64)).astype(np.float32)

    W1h = W1.astype(np.float16)
    W2h = W2.astype(np.float16)
    W3h = Wso.astype(np.float16)

    wpack = np.zeros((128, 312), np.float16)
    for c8 in range(8):                             # L1: 8-chunk block-diag
        wpack[12 * c8:12 * c8 + 12,
              W1OFF + 16 * c8:W1OFF + 16 * c8 + 16] = W1h
    for c8 in range(8):                             # L2: 8-chunk block-diag
        wpack[16 * c8:16 * c8 + 16,
              W2OFF + 8 * c8:W2OFF + 8 * c8 + 8] = W2h
    w3b = np.zeros((128, 48), np.float16)           # L3: 16-chunk block-diag
    for e in range(2):
        for c8 in range(8):
            t = 8 * e + c8
            w3b[64 * e + 8 * c8:64 * e + 8 * c8 + 8,
                3 * t:3 * t + 3] = W3h
    wpack[:, W3OFF:W3OFF + 48] = w3b
    wpack[:, W3OFF + 64:W3OFF + 112] = w3b

    bias32 = np.zeros((128, 4), np.float32)
    bias32[:, 0] = np.tile(b1, 8)
    bias32[:, 1] = np.tile(b2, 16)
    bias32[0:48, 2] = np.tile(bso, 16)
    bias32[64:112, 2] = np.tile(bso, 16)
    wpack[:, 304:312] = bias32.view(np.float16)

    in_maps = []
    for c in range(N_CORES):
        qc = q[c * BPC:(c + 1) * BPC].astype(np.float16)
        # chunk c = 8p + c8 (512 atoms); [row = 12*c8 + f, col = 512p + a]
        t = qc.reshape(NP, 8, TN, F)                # p c8 a f
        xc = np.ascontiguousarray(
            t.transpose(1, 3, 0, 2).reshape(96, 4096))
        in_maps.append({"xin": xc, "wpack": wpack})
    return in_maps, force_prev


def _host_gather(results):
    out = np.empty((B, 3), np.float32)
    for c in range(N_CORES):
        Oc = results[c]["out"]                      # [112, 1024] fp16
        oc = np.empty((64, TN, 3), np.float32)
        for r in range(2):
            for m in range(2):
                blkO = Oc[64 * m:64 * m + 48,
                          TN * r:TN * r + TN].astype(np.float32)
                # row = 3t + rr, t = 8e + c8; chunk = 32r + 16m + t
                oc[32 * r + 16 * m:32 * r + 16 * m + 16] = (
                    blkO.reshape(16, 3, TN).transpose(0, 2, 1))
        out[c * BPC:(c + 1) * BPC] = oc.reshape(BPC, 3)
    return out


LAST_RESULT = None


def kernel(x, W1, b1, W2, b2, Ws, bs, Wo, bo):
    from concourse.bass_utils import run_bass_kernel_spmd

    in_maps, force_prev = _host_prep(x, W1, b1, W2, b2, Ws, bs, Wo, bo)
    nc = _build_nc()
    res = run_bass_kernel_spmd(nc, in_maps, core_ids=list(range(N_CORES)))
    globals()["LAST_RESULT"] = res
    out = _host_gather(res.results)
    return (out, force_prev)
